# revision 1
# baseline (speedup 1.0000x reference)
"""Multi-head attention (RoPE, causal) Bass kernel for 8 TRN2 NeuronCores.

Sharding: 2-way batch x 4-way heads (4 heads per core).
Per-core inputs (DRAM, float32r unless noted):
  xT   [1024, 2048]  x[b].T
  wq/wk/wv [1024, 256]  per-head-group columns
  wo   [256, 1024]   per-head-group rows
  cdup/sdup [128, 2048] float32 RoPE tables (duplicated per stacked head pair)
  p64  [128, 128]    rotate-half partition permutation
  tri  [128, 128]    causal mask for diagonal blocks (j <= i)
  onesc [128, 65]    ones
Output: out [2048, 1024] partial (summed over the 4 head-group cores on host).

Layout notes:
  QT/KT stacked [128, S]: partitions 0-63 head even, 64-127 head odd, f32r.
  Scores computed transposed: ST[j-chunk 128, i 512] per head; softmax
  denominator comes free from an appended ones-column on V (PSUM row 64).
  Normalization: DVE reciprocal of the l-row -> gpsimd partition_broadcast
  -> DVE multiply; deferred by one i-slice so the PE never waits on it.
"""
import numpy as np
from contextlib import ExitStack

import concourse.bass as bass
import concourse.tile as tile
from concourse import bacc, mybir
from concourse.bass_utils import run_bass_kernel_spmd

D_IN = 1024
D_OUT = 1024
HD = 64                   # head dim
S = 2048                  # sequence length
B = 2
THETA = 10000.0
NCORES = 8
IS = 512                  # i-slice width
NIS = S // IS             # 4 i-slices
NJC = S // 128            # 16 j-chunks

F32 = mybir.dt.float32
F32R = mybir.dt.float32r


def build_kernel():
    nc = bacc.Bacc("TRN2", target_bir_lowering=False, debug=False)

    # host pre-shuffled so every DMA is contiguous per partition:
    # xtr[it, p, c, i] = x[b]^T[128c+p, 512it+i]; w*r[p, c, n] = W[128c+p, n]
    xT = nc.dram_tensor("xT", [NIS, 128, 8, IS], F32R, kind="ExternalInput").ap()
    wq = nc.dram_tensor("wq", [128, 8, 256], F32R, kind="ExternalInput").ap()
    wk = nc.dram_tensor("wk", [128, 8, 256], F32R, kind="ExternalInput").ap()
    wv = nc.dram_tensor("wv", [128, 8, 256], F32R, kind="ExternalInput").ap()
    wo = nc.dram_tensor("wo", [128, 2, 1024], F32R, kind="ExternalInput").ap()
    cdup = nc.dram_tensor("cdup", [128, S], F32, kind="ExternalInput").ap()
    sdup = nc.dram_tensor("sdup", [128, S], F32, kind="ExternalInput").ap()
    p64 = nc.dram_tensor("p64", [128, 128], F32R, kind="ExternalInput").ap()
    tri = nc.dram_tensor("tri", [128, 128], F32R, kind="ExternalInput").ap()
    onesc = nc.dram_tensor("onesc", [128, 65], F32R, kind="ExternalInput").ap()
    out = nc.dram_tensor("out", [S, D_OUT], F32, kind="ExternalOutput").ap()

    with tile.TileContext(nc) as tc, ExitStack() as ctx:
        singles = ctx.enter_context(tc.tile_pool(name="singles", bufs=1))
        xpool = ctx.enter_context(tc.tile_pool(name="xpool", bufs=2))
        rope_tmp = ctx.enter_context(tc.tile_pool(name="rope_tmp", bufs=3))
        expp = ctx.enter_context(tc.tile_pool(name="expp", bufs=3))
        bcp = ctx.enter_context(tc.tile_pool(name="bcp", bufs=2))
        ctxp = ctx.enter_context(tc.tile_pool(name="ctxp", bufs=2))
        outp = ctx.enter_context(tc.tile_pool(name="outp", bufs=3))
        # PSUM: ps_a 4x1 bank + ps_b 2x2 banks = 8 banks
        ps_a = ctx.enter_context(tc.tile_pool(name="ps_a", bufs=4, space="PSUM"))
        ps_b = ctx.enter_context(tc.tile_pool(name="ps_b", bufs=2, space="PSUM"))

        # ---- weights / tables, ordered by first use ----
        # (wq/wk/xt0 gate the very first matmuls; wv/tri later; wo last)
        def w_dma(name, ap, split=1):
            # split per chunk-group so the first matmuls wait on less data
            t = singles.tile([128, 8, 256], F32R, tag=name, name=name)
            step = 8 // split
            for s in range(split):
                nc.sync.dma_start(out=t[:, s * step:(s + 1) * step, :],
                                  in_=ap[:, s * step:(s + 1) * step, :])
            return t

        xts = {}

        def xt_dma(it, split=1):
            t = xpool.tile([128, 8, IS], F32R, tag="xt", name=f"xt{it}")
            for s in range(0, 8, 8 // split):
                nc.sync.dma_start(out=t[:, s:s + 8 // split, :],
                                  in_=xT[it, :, s:s + 8 // split, :])
            xts[it] = t

        # interleave wq/xt0 chunk DMAs: the first projection chain consumes
        # (wq chunk c, xt0 chunk c) in order
        wq_t = singles.tile([128, 8, 256], F32R, tag="wq", name="wq")
        xt0_t = xpool.tile([128, 8, IS], F32R, tag="xt", name="xt0")
        for c in range(8):
            nc.sync.dma_start(out=wq_t[:, c, :], in_=wq[:, c, :])
            nc.sync.dma_start(out=xt0_t[:, c, :], in_=xT[0, :, c, :])
        xts[0] = xt0_t
        w_sb = {"wq": wq_t, "wk": w_dma("wk", wk, split=4)}
        c_sb = singles.tile([128, S], F32, tag="cdup")
        nc.sync.dma_start(out=c_sb, in_=cdup)
        s_sb = singles.tile([128, S], F32, tag="sdup")
        nc.sync.dma_start(out=s_sb, in_=sdup)
        p64_sb = singles.tile([128, 128], F32R, tag="p64")
        nc.sync.dma_start(out=p64_sb, in_=p64)
        w_sb["wv"] = w_dma("wv", wv)
        onesc_sb = singles.tile([128, 65], F32R, tag="ones")
        nc.sync.dma_start(out=onesc_sb, in_=onesc)
        tri_sb = singles.tile([128, 128], F32R, tag="tri")
        nc.sync.dma_start(out=tri_sb, in_=tri)
        xt_dma(1)
        # deferred: wo is not needed until the first out-projection
        wo_sb = singles.tile([128, 2, 1024], F32R, tag="wo")

        # persistent SBUF state
        qt = [singles.tile([128, S], F32R, tag=f"qt{p}", name=f"qt{p}")
              for p in range(2)]
        kt = [singles.tile([128, S], F32R, tag=f"kt{p}", name=f"kt{p}")
              for p in range(2)]
        v4 = singles.tile([128, NJC, 4, 65], F32R, tag="v4")
        v4_ones = bass.AP(tensor=v4.tensor, offset=64,
                          ap=[[NJC * 4 * 65, 128], [65, NJC * 4]])
        nc.vector.tensor_copy(v4_ones, onesc_sb[:, 0:64])
        # 1/l rows at partition 0: [head, it parity, i]
        rl = singles.tile([128, 4, 2, IS], F32, tag="rl")

        # ================= Phase 1: QKV projections + RoPE =================
        for it in range(NIS):
            xt_t = xts[it]
            for tname, wt, dests in (("q", w_sb["wq"], qt), ("k", w_sb["wk"], kt)):
                for p in range(2):
                    proj = ps_a.tile([128, IS], F32, tag="a", name="proj")
                    for c in range(8):
                        nc.tensor.matmul(proj, wt[:, c, 128 * p:128 * (p + 1)],
                                         xt_t[:, c, :],
                                         start=(c == 0), stop=(c == 7))
                    # RoPE: rot = proj * cos + (P64 @ proj) * sin'
                    raw = rope_tmp.tile([128, IS], F32R, tag="raw")
                    nc.scalar.copy(raw, proj)
                    perm = ps_b.tile([128, 2, IS], F32, tag="b", name="perm")
                    nc.tensor.matmul(perm[:, 0, :], p64_sb, raw,
                                     start=True, stop=True)
                    t1 = rope_tmp.tile([128, IS], F32, tag="t1")
                    nc.vector.tensor_mul(t1, proj, c_sb[:, it * IS:(it + 1) * IS])
                    t2 = rope_tmp.tile([128, IS], F32, tag="t2")
                    nc.vector.tensor_mul(t2, perm[:, 0, :],
                                         s_sb[:, it * IS:(it + 1) * IS])
                    nc.vector.tensor_add(
                        dests[p][:, it * IS:(it + 1) * IS], t1, t2)

            # V projection: [j, 256] tiles, 4 j-subtiles per i-slice
            for half in range(2):
                vps = ps_b.tile([128, 2, 256], F32, tag="b", name="vps")
                for js in range(2):
                    jt = it * 4 + half * 2 + js
                    for c in range(8):
                        nc.tensor.matmul(
                            vps[:, js, :],
                            xt_t[:, c, 128 * (half * 2 + js):128 * (half * 2 + js + 1)],
                            w_sb["wv"][:, c, :],
                            start=(c == 0), stop=(c == 7))
                for js in range(2):
                    jt = it * 4 + half * 2 + js
                    nc.vector.tensor_copy(v4[:, jt, :, 0:64],
                                          vps[:, js, :].rearrange("p (h d) -> p h d", h=4))
            if it == 0:
                nc.sync.dma_start(out=wo_sb, in_=wo)
            if it + 2 < NIS:
                xt_dma(it + 2)

        # ============ Phase 2: attention, norm/out-proj deferred 1 it ======
        def attention_head(pair, half, it):
            h = 2 * pair + half
            hb = 64 * half
            qs = qt[pair][hb:hb + 64, :]
            ks = kt[pair][hb:hb + 64, :]
            njc = 4 * it + 4
            ctx_ps = ps_a.tile([128, IS], F32, tag="a", name=f"ctx{h}")
            for q0 in range(0, njc, 2):
                nq = min(2, njc - q0)
                quad = ps_b.tile([128, 2, IS], F32, tag="b", name="quad")
                exps = expp.tile([128, 2, IS], F32R, tag="e", name="exps")
                for qi in range(nq):
                    jc = q0 + qi
                    c0 = max(0, 128 * (jc - 4 * it))
                    nc.tensor.matmul(
                        quad[:, qi, c0:IS],
                        ks[:, 128 * jc:128 * (jc + 1)],
                        qs[:, it * IS + c0:(it + 1) * IS],
                        start=True, stop=True)
                nc.scalar.activation(
                    exps[:, 0:nq, :], quad[:, 0:nq, :],
                    mybir.ActivationFunctionType.Exp, scale=0.125)
                for qi in range(nq):
                    jc = q0 + qi
                    c0 = max(0, 128 * (jc - 4 * it))
                    if c0 > 0 or jc == 4 * it:
                        # causal mask of the diagonal block
                        nc.vector.tensor_mul(
                            exps[:, qi, c0:c0 + 128],
                            exps[:, qi, c0:c0 + 128], tri_sb)
                    nc.tensor.matmul(
                        ctx_ps[0:65, c0:IS],
                        v4[:, jc, h, :],
                        exps[:, qi, c0:IS],
                        start=(jc == 0), stop=(jc == njc - 1))
            return ctx_ps

        def recip_l(ctx_ps, h, it):
            # 1/l of row 64, written to partition 0 (DVE iterative divide,
            # ~3.4us; runs in the shadow of the next head's attention)
            with nc.allow_low_precision(reason="fp32 width"):
                nc.vector.reciprocal(rl[0:1, h, it % 2, :],
                                     ctx_ps[64:65, :])

        def norm_and_outproj(ctx_tiles, it):
            ctxs_pair = []
            for pair in range(2):
                ctxs = ctxp.tile([128, IS], F32R, tag="c", name="ctxs")
                for half in range(2):
                    bcs = bcp.tile([64, IS], F32, tag="bc", name="bcs")
                    nc.gpsimd.partition_broadcast(
                        bcs, rl[0:1, 2 * pair + half, it % 2, :])
                    nc.vector.tensor_mul(
                        ctxs[64 * half:64 * half + 64, :],
                        ctx_tiles[2 * pair + half][0:64, :], bcs)
                ctxs_pair.append(ctxs)
            for ib in range(4):
                ot = outp.tile([128, 1024], F32, tag="o", name="ot")
                for nt in range(2):
                    ops = ps_a.tile([128, IS], F32, tag="a", name="ops")
                    for pair in range(2):
                        nc.tensor.matmul(
                            ops,
                            ctxs_pair[pair][:, 128 * ib:128 * (ib + 1)],
                            wo_sb[:, pair, nt * IS:(nt + 1) * IS],
                            start=(pair == 0), stop=(pair == 1))
                    if nt == 0:
                        nc.vector.tensor_copy(ot[:, 0:IS], ops)
                    else:
                        nc.scalar.copy(ot[:, IS:1024], ops)
                nc.sync.dma_start(
                    out=out[it * IS + 128 * ib: it * IS + 128 * (ib + 1), :],
                    in_=ot)

        # Issue order: norm(it-1) goes after head0(it)'s matmuls but BEFORE
        # recip(h0, it), so on the DVE FIFO the norm multiplies are not
        # queued behind a fresh 3.4us reciprocal.
        pending = None
        for it in range(NIS):
            ctx_tiles = {}
            ctx_tiles[0] = attention_head(0, 0, it)
            if pending is not None:
                norm_and_outproj(*pending)
            recip_l(ctx_tiles[0], 0, it)
            for h, (pair, half) in enumerate([(0, 1), (1, 0), (1, 1)], start=1):
                ctx_tiles[h] = attention_head(pair, half, it)
                recip_l(ctx_tiles[h], h, it)
            pending = (ctx_tiles, it)
        norm_and_outproj(*pending)

    nc.compile()
    return nc


def _host_tables():
    inv_freq = 1.0 / (THETA ** (np.arange(0, HD, 2, dtype=np.float64) / HD))
    pos = np.arange(S, dtype=np.float64)
    ang = pos[None, :] * inv_freq[:, None]          # [32, S]
    cos32 = np.cos(ang).astype(np.float32)
    sin32 = np.sin(ang).astype(np.float32)
    cdup = np.concatenate([cos32, cos32, cos32, cos32], axis=0)  # [128, S]
    s_signed = np.concatenate([-sin32, sin32, -sin32, sin32], axis=0)
    p64 = np.zeros((128, 128), dtype=np.float32)
    for m in range(128):
        blk = m - (m % 64)
        d = m % 64
        p64[blk + ((d + 32) % 64), m] = 1.0
    tri = (np.arange(128)[:, None] <= np.arange(128)[None, :]).astype(np.float32)
    return cdup, s_signed, p64, tri


_NC_CACHE = {}


def make_in_maps(x, W_q, W_k, W_v, W_o):
    cdup, sdup, p64, tri = _host_tables()
    ones = np.ones((128, 65), dtype=np.float32)
    def wshuf(w):  # [1024, 256] -> [128, 8, 256]
        return np.ascontiguousarray(w.reshape(8, 128, 256).transpose(1, 0, 2))

    in_maps = []
    for c in range(NCORES):
        b, g = divmod(c, 4)
        cols = slice(256 * g, 256 * (g + 1))
        # xtr[it, p, ch, i] = x[b][512it+i, 128ch+p]
        xtr = np.ascontiguousarray(
            x[b].reshape(NIS, IS, 8, 128).transpose(0, 3, 2, 1))
        in_maps.append({
            "xT": xtr,
            "wq": wshuf(W_q[:, cols]),
            "wk": wshuf(W_k[:, cols]),
            "wv": wshuf(W_v[:, cols]),
            "wo": np.ascontiguousarray(
                W_o[cols, :].reshape(2, 128, 1024).transpose(1, 0, 2)),
            "cdup": cdup, "sdup": sdup, "p64": p64, "tri": tri,
            "onesc": ones,
        })
    return in_maps


def kernel(x, W_q, W_k, W_v, W_o):
    x = np.ascontiguousarray(x, dtype=np.float32)
    W_q = np.ascontiguousarray(W_q, dtype=np.float32)
    W_k = np.ascontiguousarray(W_k, dtype=np.float32)
    W_v = np.ascontiguousarray(W_v, dtype=np.float32)
    W_o = np.ascontiguousarray(W_o, dtype=np.float32)

    if "nc" not in _NC_CACHE:
        _NC_CACHE["nc"] = build_kernel()
    nc = _NC_CACHE["nc"]

    in_maps = make_in_maps(x, W_q, W_k, W_v, W_o)
    res = run_bass_kernel_spmd(nc, in_maps, list(range(NCORES)))
    outs = [res.results[c]["out"] for c in range(NCORES)]
    full = np.empty((B, S, D_OUT), dtype=np.float32)
    for b in range(B):
        full[b] = outs[4 * b] + outs[4 * b + 1] + outs[4 * b + 2] + outs[4 * b + 3]
    return full



# revision 13
# speedup vs baseline: 1.2236x; 1.2236x over previous
"""Multi-head attention (RoPE, causal) Bass kernel for 8 TRN2 NeuronCores.

Sharding: 2-way batch x 4-way heads (4 heads per core).
Per-core inputs (DRAM, float32r unless noted):
  xT   [1024, 2048]  x[b].T
  wq/wk/wv [1024, 256]  per-head-group columns
  wo   [256, 1024]   per-head-group rows
  cdup/sdup [128, 2048] float32 RoPE tables (duplicated per stacked head pair)
  p64  [128, 128]    rotate-half partition permutation
  tri  [128, 128]    causal mask for diagonal blocks (j <= i)
  onesc [128, 65]    ones
Output: out [2048, 1024] partial (summed over the 4 head-group cores on host).

Layout notes:
  QT/KT stacked [128, S]: partitions 0-63 head even, 64-127 head odd, f32r.
  Scores computed transposed: ST[j-chunk 128, i 512] per head; softmax
  denominator comes free from an appended ones-column on V (PSUM row 64).
  Phase 2 runs a flat software pipeline over (head, chunk-pair): scores ->
  exp (Act) -> mask (DVE) -> ctx matmul deferred one pair so the PE never
  waits on the exp; out-proj matmuls of the previous i-slice are pulled
  into the gaps.  1/l via custom-DVE fast reciprocal (~0.7us).
"""
import numpy as np
from contextlib import ExitStack

import concourse.bass as bass
import concourse.tile as tile
from concourse import bacc, mybir
from concourse.bass_utils import run_bass_kernel_spmd

D_IN = 1024
D_OUT = 1024
HD = 64                   # head dim
S = 2048                  # sequence length
B = 2
THETA = 10000.0
NCORES = 8
IS = 512                  # i-slice width
NIS = S // IS             # 4 i-slices
NJC = S // 128            # 16 j-chunks

F32 = mybir.dt.float32
F32R = mybir.dt.float32r


def build_kernel():
    nc = bacc.Bacc("TRN2", target_bir_lowering=False, debug=False)

    # host pre-shuffled so every DMA is contiguous per partition:
    # xtr[it, p, c, i] = x[b]^T[128c+p, 512it+i]; w*r[p, c, n] = W[128c+p, n]
    xT = nc.dram_tensor("xT", [NIS, 128, 8, IS], F32R, kind="ExternalInput").ap()
    wq = nc.dram_tensor("wq", [128, 8, 256], F32R, kind="ExternalInput").ap()
    wk = nc.dram_tensor("wk", [128, 8, 256], F32R, kind="ExternalInput").ap()
    wv = nc.dram_tensor("wv", [128, 8, 256], F32R, kind="ExternalInput").ap()
    wo = nc.dram_tensor("wo", [128, 2, 1024], F32R, kind="ExternalInput").ap()
    cdup = nc.dram_tensor("cdup", [128, S], F32, kind="ExternalInput").ap()
    sdup = nc.dram_tensor("sdup", [128, S], F32, kind="ExternalInput").ap()
    p64 = nc.dram_tensor("p64", [128, 128], F32R, kind="ExternalInput").ap()
    tri = nc.dram_tensor("tri", [128, 128], F32R, kind="ExternalInput").ap()
    onesc = nc.dram_tensor("onesc", [128, 65], F32R, kind="ExternalInput").ap()
    out = nc.dram_tensor("out", [S, D_OUT], F32, kind="ExternalOutput").ap()

    with tile.TileContext(nc) as tc, ExitStack() as ctx:
        singles = ctx.enter_context(tc.tile_pool(name="singles", bufs=1))
        xpool = ctx.enter_context(tc.tile_pool(name="xpool", bufs=2))
        rope_tmp = ctx.enter_context(tc.tile_pool(name="rope_tmp", bufs=3))
        expp = ctx.enter_context(tc.tile_pool(name="expp", bufs=3))
        bcp = ctx.enter_context(tc.tile_pool(name="bcp", bufs=2))
        ctxp = ctx.enter_context(tc.tile_pool(name="ctxp", bufs=2))
        outp = ctx.enter_context(tc.tile_pool(name="outp", bufs=3))
        # PSUM budget (8 banks): tag "b" 2x[128,2,512] = 4 banks (quads +
        # phase-1 vps), tag "cx" 2x[128,512] = 2 banks (proj chains + ctx
        # accumulators), tag "op" 2x[128,512] = 2 banks (perm + out-proj).
        psum = ctx.enter_context(tc.tile_pool(name="psum", bufs=2, space="PSUM"))

        # ---- weights / tables, ordered by first use ----
        def w_dma(name, ap, split=1):
            t = singles.tile([128, 8, 256], F32R, tag=name, name=name)
            step = 8 // split
            for s in range(split):
                nc.sync.dma_start(out=t[:, s * step:(s + 1) * step, :],
                                  in_=ap[:, s * step:(s + 1) * step, :])
            return t

        xts = {}

        def xt_dma(it, split=1):
            t = xpool.tile([128, 8, IS], F32R, tag="xt", name=f"xt{it}")
            for s in range(0, 8, 8 // split):
                nc.sync.dma_start(out=t[:, s:s + 8 // split, :],
                                  in_=xT[it, :, s:s + 8 // split, :])
            xts[it] = t

        # interleave wq/xt0 chunk DMAs: the first projection chain consumes
        # (wq chunk c, xt0 chunk c) in order
        wq_t = singles.tile([128, 8, 256], F32R, tag="wq", name="wq")
        xt0_t = xpool.tile([128, 8, IS], F32R, tag="xt", name="xt0")
        for c in range(8):
            nc.sync.dma_start(out=wq_t[:, c, :], in_=wq[:, c, :])
            nc.sync.dma_start(out=xt0_t[:, c, :], in_=xT[0, :, c, :])
        xts[0] = xt0_t
        w_sb = {"wq": wq_t, "wk": w_dma("wk", wk, split=4)}
        c_sb = singles.tile([128, S], F32, tag="cdup")
        nc.sync.dma_start(out=c_sb, in_=cdup)
        s_sb = singles.tile([128, S], F32, tag="sdup")
        nc.sync.dma_start(out=s_sb, in_=sdup)
        p64_sb = singles.tile([128, 128], F32R, tag="p64")
        nc.sync.dma_start(out=p64_sb, in_=p64)
        w_sb["wv"] = w_dma("wv", wv)
        onesc_sb = singles.tile([128, 65], F32R, tag="ones")
        nc.sync.dma_start(out=onesc_sb, in_=onesc)
        tri_sb = singles.tile([128, 128], F32R, tag="tri")
        nc.sync.dma_start(out=tri_sb, in_=tri)
        xt_dma(1)
        # deferred: wo is not needed until the first out-projection
        wo_sb = singles.tile([128, 2, 1024], F32R, tag="wo")

        # persistent SBUF state
        qt = [singles.tile([128, S], F32R, tag=f"qt{p}", name=f"qt{p}")
              for p in range(2)]
        kt = [singles.tile([128, S], F32R, tag=f"kt{p}", name=f"kt{p}")
              for p in range(2)]
        v4 = singles.tile([128, NJC, 4, 65], F32R, tag="v4")
        v4_ones = bass.AP(tensor=v4.tensor, offset=64,
                          ap=[[NJC * 4 * 65, 128], [65, NJC * 4]])
        nc.vector.tensor_copy(v4_ones, onesc_sb[:, 0:64])
        # 1/l rows, all on partition 0: [h, i] along the free dim.
        # lrow stages the PSUM row-64 l to partition 0 first: the custom-DVE
        # reciprocal mishandles nonzero input partition offsets.
        rl = singles.tile([128, 4, IS], F32, tag="rl")
        lrow = singles.tile([128, 4, IS], F32, tag="lrow")

        # ================= Phase 1: QKV projections + RoPE =================
        # Per (tensor, pair): 8-matmul projection chain; the rotate-half perm
        # matmul is deferred one chain so the PE never waits on the PSUM->SBUF
        # copy (Act).  t1 (proj*cos) issues immediately (frees PSUM earlier);
        # t2+add wait for the perm.
        def rope_stage1(proj, key, it):
            # raw copy for the perm matmul input + direct cos term
            raw = rope_tmp.tile([128, IS], F32R, tag="raw", name=f"raw{key}")
            nc.scalar.copy(raw, proj)
            t1 = rope_tmp.tile([128, IS], F32, tag="t1", name=f"t1{key}")
            nc.vector.tensor_mul(t1, proj, c_sb[:, it * IS:(it + 1) * IS])
            return raw, t1

        def rope_stage2(raw, t1, dest, it):
            perm = psum.tile([128, IS], F32, tag="op", name="perm")
            nc.tensor.matmul(perm, p64_sb, raw, start=True, stop=True)
            t2 = rope_tmp.tile([128, IS], F32, tag="t2")
            nc.vector.tensor_mul(t2, perm, s_sb[:, it * IS:(it + 1) * IS])
            nc.vector.tensor_add(dest, t1, t2)

        for it in range(NIS):
            xt_t = xts[it]
            pending_rope = None
            for tname, wt, dests in (("q", w_sb["wq"], qt), ("k", w_sb["wk"], kt)):
                for p in range(2):
                    proj = psum.tile([128, IS], F32, tag="cx", name="proj")
                    for c in range(8):
                        nc.tensor.matmul(proj, wt[:, c, 128 * p:128 * (p + 1)],
                                         xt_t[:, c, :],
                                         start=(c == 0), stop=(c == 7))
                    if pending_rope is not None:
                        rope_stage2(*pending_rope)
                    raw, t1 = rope_stage1(proj, f"{tname}{p}", it)
                    pending_rope = (raw, t1,
                                    dests[p][:, it * IS:(it + 1) * IS], it)

            # V projection: [j, 256] tiles, 4 j-subtiles per i-slice
            for half in range(2):
                vps = psum.tile([128, 2, 256], F32, tag="b", name="vps")
                for js in range(2):
                    for c in range(8):
                        nc.tensor.matmul(
                            vps[:, js, :],
                            xt_t[:, c, 128 * (half * 2 + js):128 * (half * 2 + js + 1)],
                            w_sb["wv"][:, c, :],
                            start=(c == 0), stop=(c == 7))
                if half == 0 and pending_rope is not None:
                    rope_stage2(*pending_rope)
                    pending_rope = None
                for js in range(2):
                    jt = it * 4 + half * 2 + js
                    nc.vector.tensor_copy(v4[:, jt, :, 0:64],
                                          vps[:, js, :].rearrange("p (h d) -> p h d", h=4))
            if it == 0:
                nc.sync.dma_start(out=wo_sb, in_=wo)
            if it + 2 < NIS:
                xt_dma(it + 2)

        # ========== Phase 2: flat pipelined attention + out-proj ===========
        def outproj_gen(ctxs_pair, it):
            """Yields once per PE step (2 matmuls); 8 steps per i-slice."""
            for ib in range(4):
                ot = outp.tile([128, 1024], F32, tag="o", name="ot")
                for nt in range(2):
                    ops = psum.tile([128, IS], F32, tag="op", name="ops")
                    for pair in range(2):
                        nc.tensor.matmul(
                            ops,
                            ctxs_pair[pair][:, 128 * ib:128 * (ib + 1)],
                            wo_sb[:, pair, nt * IS:(nt + 1) * IS],
                            start=(pair == 0), stop=(pair == 1))
                    nc.vector.tensor_copy(ot[:, nt * IS:(nt + 1) * IS], ops)
                    yield
                nc.sync.dma_start(
                    out=out[it * IS + 128 * ib: it * IS + 128 * (ib + 1), :],
                    in_=ot)

        ctxs_pair = None
        outgen = iter(())

        for it in range(NIS):
            njc = 4 * it + 4
            new_ctxs = [None, None]
            ctx_ps = {}
            pend = []

            def drain(it=it, njc=njc, new_ctxs=new_ctxs, ctx_ps=ctx_ps):
                h, q0, exps = pend.pop(0)
                for qi in range(2):
                    jc = q0 + qi
                    c0 = max(0, 128 * (jc - 4 * it))
                    nc.tensor.matmul(
                        ctx_ps[h][0:65, c0:IS],
                        v4[:, jc, h, :],
                        exps[:, qi, c0:IS],
                        start=(jc == 0), stop=(jc == njc - 1))
                if q0 + 2 == njc:
                    # head complete: 1/l, broadcast, normalize -> ctxs SBUF
                    pair, half = divmod(h, 2)
                    nc.vector.tensor_copy(lrow[0:1, h, :], ctx_ps[h][64:65, :])
                    nc.vector.reciprocal_approx_fast(rl[0:1, h, :],
                                                     lrow[0:1, h, :])
                    if half == 0:
                        new_ctxs[pair] = ctxp.tile([128, IS], F32R,
                                                   tag="c", name="ctxs")
                    bcs = bcp.tile([64, IS], F32, tag="bc", name="bcs")
                    nc.gpsimd.partition_broadcast(bcs, rl[0:1, h, :])
                    nc.vector.tensor_mul(
                        new_ctxs[pair][64 * half:64 * half + 64, :],
                        ctx_ps[h][0:64, :], bcs)

            for h in range(4):
                pair, half = divmod(h, 2)
                hb = 64 * half
                qs = qt[pair][hb:hb + 64, :]
                ks = kt[pair][hb:hb + 64, :]
                ctx_ps[h] = psum.tile([128, IS], F32, tag="cx", name=f"ctx{h}")
                for q0 in range(0, njc, 2):
                    quad = psum.tile([128, 2, IS], F32, tag="b", name="quad")
                    exps = expp.tile([128, 2, IS], F32R, tag="e", name="exps")
                    trim0 = max(0, 128 * (q0 - 4 * it))
                    for qi in range(2):
                        jc = q0 + qi
                        c0 = max(0, 128 * (jc - 4 * it))
                        nc.tensor.matmul(
                            quad[:, qi, c0:IS],
                            ks[:, 128 * jc:128 * (jc + 1)],
                            qs[:, it * IS + c0:(it + 1) * IS],
                            start=True, stop=True)
                    nc.scalar.activation(
                        exps[:, 0:2, trim0:IS], quad[:, 0:2, trim0:IS],
                        mybir.ActivationFunctionType.Exp, scale=0.125)
                    for qi in range(2):
                        jc = q0 + qi
                        c0 = max(0, 128 * (jc - 4 * it))
                        if c0 > 0 or jc == 4 * it:
                            # causal mask of the diagonal block
                            nc.vector.tensor_mul(
                                exps[:, qi, c0:c0 + 128],
                                exps[:, qi, c0:c0 + 128], tri_sb)
                    next(outgen, None)
                    if pend:
                        drain()
                    pend.append((h, q0, exps))
            while pend:
                drain()
            for _ in outgen:
                pass
            ctxs_pair = new_ctxs
            outgen = outproj_gen(ctxs_pair, it)
        for _ in outgen:
            pass

    nc.compile()
    return nc


def _host_tables():
    inv_freq = 1.0 / (THETA ** (np.arange(0, HD, 2, dtype=np.float64) / HD))
    pos = np.arange(S, dtype=np.float64)
    ang = pos[None, :] * inv_freq[:, None]          # [32, S]
    cos32 = np.cos(ang).astype(np.float32)
    sin32 = np.sin(ang).astype(np.float32)
    cdup = np.concatenate([cos32, cos32, cos32, cos32], axis=0)  # [128, S]
    s_signed = np.concatenate([-sin32, sin32, -sin32, sin32], axis=0)
    p64 = np.zeros((128, 128), dtype=np.float32)
    for m in range(128):
        blk = m - (m % 64)
        d = m % 64
        p64[blk + ((d + 32) % 64), m] = 1.0
    tri = (np.arange(128)[:, None] <= np.arange(128)[None, :]).astype(np.float32)
    return cdup, s_signed, p64, tri


_NC_CACHE = {}


def make_in_maps(x, W_q, W_k, W_v, W_o):
    cdup, sdup, p64, tri = _host_tables()
    ones = np.ones((128, 65), dtype=np.float32)
    def wshuf(w):  # [1024, 256] -> [128, 8, 256]
        return np.ascontiguousarray(w.reshape(8, 128, 256).transpose(1, 0, 2))

    in_maps = []
    for c in range(NCORES):
        b, g = divmod(c, 4)
        cols = slice(256 * g, 256 * (g + 1))
        # xtr[it, p, ch, i] = x[b][512it+i, 128ch+p]
        xtr = np.ascontiguousarray(
            x[b].reshape(NIS, IS, 8, 128).transpose(0, 3, 2, 1))
        in_maps.append({
            "xT": xtr,
            "wq": wshuf(W_q[:, cols]),
            "wk": wshuf(W_k[:, cols]),
            "wv": wshuf(W_v[:, cols]),
            "wo": np.ascontiguousarray(
                W_o[cols, :].reshape(2, 128, 1024).transpose(1, 0, 2)),
            "cdup": cdup, "sdup": sdup, "p64": p64, "tri": tri,
            "onesc": ones,
        })
    return in_maps


def kernel(x, W_q, W_k, W_v, W_o):
    x = np.ascontiguousarray(x, dtype=np.float32)
    W_q = np.ascontiguousarray(W_q, dtype=np.float32)
    W_k = np.ascontiguousarray(W_k, dtype=np.float32)
    W_v = np.ascontiguousarray(W_v, dtype=np.float32)
    W_o = np.ascontiguousarray(W_o, dtype=np.float32)

    if "nc" not in _NC_CACHE:
        _NC_CACHE["nc"] = build_kernel()
    nc = _NC_CACHE["nc"]

    in_maps = make_in_maps(x, W_q, W_k, W_v, W_o)
    res = run_bass_kernel_spmd(nc, in_maps, list(range(NCORES)))
    outs = [res.results[c]["out"] for c in range(NCORES)]
    full = np.empty((B, S, D_OUT), dtype=np.float32)
    for b in range(B):
        full[b] = outs[4 * b] + outs[4 * b + 1] + outs[4 * b + 2] + outs[4 * b + 3]
    return full


# revision 14
# speedup vs baseline: 1.3300x; 1.0870x over previous
"""Multi-head attention (RoPE, causal) Bass kernel for 8 TRN2 NeuronCores.

Sharding: 2-way batch x 4-way heads (4 heads per core); per-core partial
out[2048, 1024] summed on host (replaces the W_o-row-parallel AllReduce).

Compute layout (per core):
  Phase 1: QKV projections (contraction over 1024 = 8 chunk matmuls into a
  PSUM bank) + RoPE: rot(q) = q*cos + (P64 @ q)*sin with the perm matmul
  deferred one chain so the PE never waits on the PSUM->SBUF staging copy.
  QT/KT stacked [128, S] bf16: partitions 0-63 head even, 64-127 head odd.
  Phase 2: flat software pipeline over (head, j-chunk-pair): scores ->
  exp (Act, pair-trimmed) -> causal mask (DVE, fused pair op) -> ctx matmul
  deferred one pair; the previous i-slice's out-projection matmuls are
  pulled into the pipeline gaps.  Softmax denominator rides as a 65th
  ones-column on V (PSUM row 64); 1/l via custom-DVE fast reciprocal off a
  partition-0 staging row.  Everything bf16 except PSUM, RoPE tables and
  the l/norm path (fp32).
"""
import numpy as np
import ml_dtypes
from contextlib import ExitStack

import concourse.bass as bass
import concourse.tile as tile
from concourse import bacc, mybir
from concourse.bass_utils import run_bass_kernel_spmd

D_IN = 1024
D_OUT = 1024
HD = 64                   # head dim
S = 2048                  # sequence length
B = 2
THETA = 10000.0
NCORES = 8
IS = 512                  # i-slice width
NIS = S // IS             # 4 i-slices
NJC = S // 128            # 16 j-chunks

F32 = mybir.dt.float32
BF16 = mybir.dt.bfloat16
BF = ml_dtypes.bfloat16


def build_kernel():
    nc = bacc.Bacc("TRN2", target_bir_lowering=False, debug=False)

    # host pre-shuffled so every DMA is contiguous per partition:
    # xT[p, it, c, i] = x[b][512it+i, 128c+p]; w*[p, c, n] = W[128c+p, n]
    xT = nc.dram_tensor("xT", [128, NIS, 8, IS], BF16, kind="ExternalInput").ap()
    wq = nc.dram_tensor("wq", [128, 8, 256], BF16, kind="ExternalInput").ap()
    wk = nc.dram_tensor("wk", [128, 8, 256], BF16, kind="ExternalInput").ap()
    wv = nc.dram_tensor("wv", [128, 8, 256], BF16, kind="ExternalInput").ap()
    wo = nc.dram_tensor("wo", [128, 2, 1024], BF16, kind="ExternalInput").ap()
    cdup = nc.dram_tensor("cdup", [128, S], F32, kind="ExternalInput").ap()
    sdup = nc.dram_tensor("sdup", [128, S], F32, kind="ExternalInput").ap()
    p64 = nc.dram_tensor("p64", [128, 128], BF16, kind="ExternalInput").ap()
    tri2 = nc.dram_tensor("tri2", [128, 2, 128], BF16, kind="ExternalInput").ap()
    onesc = nc.dram_tensor("onesc", [128, 65], BF16, kind="ExternalInput").ap()
    # out[p, it, ib, n] = row 512it+128ib+p of the [2048, 1024] partial
    out = nc.dram_tensor("out", [128, NIS, 4, 1024], BF16,
                         kind="ExternalOutput").ap()

    with tile.TileContext(nc) as tc, ExitStack() as ctx:
        singles = ctx.enter_context(tc.tile_pool(name="singles", bufs=1))
        xpool = ctx.enter_context(tc.tile_pool(name="xpool", bufs=2))
        rope_tmp = ctx.enter_context(tc.tile_pool(name="rope_tmp", bufs=3))
        expp = ctx.enter_context(tc.tile_pool(name="expp", bufs=3))
        bcp = ctx.enter_context(tc.tile_pool(name="bcp", bufs=2))
        ctxp = ctx.enter_context(tc.tile_pool(name="ctxp", bufs=2))
        outp = ctx.enter_context(tc.tile_pool(name="outp", bufs=2))
        # PSUM budget (8 banks): tag "b" 2x[128,2,512] = 4 banks (quads +
        # phase-1 vps), tag "cx" 2x[128,512] = 2 banks (proj chains + ctx
        # accumulators), tag "op" 2x[128,512] = 2 banks (perm + out-proj).
        psum = ctx.enter_context(tc.tile_pool(name="psum", bufs=2, space="PSUM"))

        # ---- weights / tables / x, ordered by first use ----
        # interleave wq/xt0 chunk DMAs: the first projection chain consumes
        # (wq chunk c, xt0 chunk c) in order
        wq_t = singles.tile([128, 8, 256], BF16, tag="wq", name="wq")
        xt0_t = xpool.tile([128, 8, IS], BF16, tag="xt", name="xt0")
        for c in range(8):
            nc.sync.dma_start(out=wq_t[:, c, :], in_=wq[:, c, :])
            nc.sync.dma_start(out=xt0_t[:, c, :], in_=xT[:, 0, c, :])
        xts = {0: xt0_t}

        def xt_dma(it):
            t = xpool.tile([128, 8, IS], BF16, tag="xt", name=f"xt{it}")
            nc.sync.dma_start(out=t, in_=xT[:, it, :, :])
            xts[it] = t

        wk_t = singles.tile([128, 8, 256], BF16, tag="wk", name="wk")
        nc.sync.dma_start(out=wk_t[:, 0:4, :], in_=wk[:, 0:4, :])
        nc.sync.dma_start(out=wk_t[:, 4:8, :], in_=wk[:, 4:8, :])
        c_sb = singles.tile([128, S], F32, tag="cdup")
        nc.sync.dma_start(out=c_sb, in_=cdup)
        p64_sb = singles.tile([128, 128], BF16, tag="p64")
        nc.sync.dma_start(out=p64_sb, in_=p64)
        s_sb = singles.tile([128, S], F32, tag="sdup")
        nc.sync.dma_start(out=s_sb, in_=sdup)
        xt_dma(1)
        wv_t = singles.tile([128, 8, 256], BF16, tag="wv", name="wv")
        nc.sync.dma_start(out=wv_t, in_=wv)
        onesc_sb = singles.tile([128, 65], BF16, tag="ones")
        nc.sync.dma_start(out=onesc_sb, in_=onesc)
        tri2_sb = singles.tile([128, 2, 128], BF16, tag="tri2")
        nc.sync.dma_start(out=tri2_sb, in_=tri2)
        # deferred: wo is not needed until the first out-projection
        wo_sb = singles.tile([128, 2, 1024], BF16, tag="wo")

        # persistent SBUF state
        qt = [singles.tile([128, S], BF16, tag=f"qt{p}", name=f"qt{p}")
              for p in range(2)]
        kt = [singles.tile([128, S], BF16, tag=f"kt{p}", name=f"kt{p}")
              for p in range(2)]
        v4 = singles.tile([128, NJC, 4, 65], BF16, tag="v4")
        v4_ones = bass.AP(tensor=v4.tensor, offset=64,
                          ap=[[NJC * 4 * 65, 128], [65, NJC * 4]])
        # 1/l rows, all on partition 0: [h, i] along the free dim.
        # lrow stages the PSUM row-64 l to partition 0 first: the custom-DVE
        # reciprocal mishandles nonzero input partition offsets.
        rl = singles.tile([128, 4, IS], F32, tag="rl")
        lrow = singles.tile([128, 4, IS], F32, tag="lrow")

        # ================= Phase 1: QKV projections + RoPE =================
        def rope_stage1(proj, key, it):
            # raw copy for the perm matmul input + direct cos term
            raw = rope_tmp.tile([128, IS], BF16, tag="raw", name=f"raw{key}")
            nc.scalar.copy(raw, proj)
            t1 = rope_tmp.tile([128, IS], F32, tag="t1", name=f"t1{key}")
            nc.vector.tensor_mul(t1, proj, c_sb[:, it * IS:(it + 1) * IS])
            return raw, t1

        def rope_stage2(raw, t1, dest, it):
            perm = psum.tile([128, IS], F32, tag="op", name="perm")
            nc.tensor.matmul(perm, p64_sb, raw, start=True, stop=True)
            t2 = rope_tmp.tile([128, IS], F32, tag="t2")
            nc.vector.tensor_mul(t2, perm, s_sb[:, it * IS:(it + 1) * IS])
            nc.vector.tensor_add(dest, t1, t2)

        for it in range(NIS):
            xt_t = xts[it]
            pending_rope = None
            for tname, wt, dests in (("q", wq_t, qt), ("k", wk_t, kt)):
                for p in range(2):
                    proj = psum.tile([128, IS], F32, tag="cx", name="proj")
                    for c in range(8):
                        nc.tensor.matmul(proj, wt[:, c, 128 * p:128 * (p + 1)],
                                         xt_t[:, c, :],
                                         start=(c == 0), stop=(c == 7))
                    if pending_rope is not None:
                        rope_stage2(*pending_rope)
                    raw, t1 = rope_stage1(proj, f"{tname}{p}", it)
                    pending_rope = (raw, t1,
                                    dests[p][:, it * IS:(it + 1) * IS], it)

            # V projection: [j, 256] tiles, 4 j-subtiles per i-slice
            for half in range(2):
                vps = psum.tile([128, 2, 256], F32, tag="b", name="vps")
                for js in range(2):
                    for c in range(8):
                        nc.tensor.matmul(
                            vps[:, js, :],
                            xt_t[:, c, 128 * (half * 2 + js):128 * (half * 2 + js + 1)],
                            wv_t[:, c, :],
                            start=(c == 0), stop=(c == 7))
                if half == 0 and pending_rope is not None:
                    rope_stage2(*pending_rope)
                    pending_rope = None
                for js in range(2):
                    jt = it * 4 + half * 2 + js
                    nc.vector.tensor_copy(v4[:, jt, :, 0:64],
                                          vps[:, js, :].rearrange("p (h d) -> p h d", h=4))
            if it == 0:
                nc.sync.dma_start(out=wo_sb, in_=wo)
            if it + 2 < NIS:
                xt_dma(it + 2)
        # ones column for the softmax denominator (deferred off the DVE
        # queue head so phase-1 RoPE is not blocked on the onesc DMA)
        nc.vector.tensor_copy(v4_ones, onesc_sb[:, 0:64])

        # ========== Phase 2: flat pipelined attention + out-proj ===========
        def outproj_gen(ctxs_pair, it):
            """Yields once per PE step (2 matmuls); 8 steps per i-slice."""
            ot = outp.tile([128, 4, 1024], BF16, tag="o", name="ot")
            for ib in range(4):
                for nt in range(2):
                    ops = psum.tile([128, IS], F32, tag="op", name="ops")
                    for pair in range(2):
                        nc.tensor.matmul(
                            ops,
                            ctxs_pair[pair][:, 128 * ib:128 * (ib + 1)],
                            wo_sb[:, pair, nt * IS:(nt + 1) * IS],
                            start=(pair == 0), stop=(pair == 1))
                    # split the PSUM->SBUF copies across Act and DVE
                    if nt == 0:
                        nc.scalar.copy(ot[:, ib, 0:IS], ops)
                    else:
                        nc.vector.tensor_copy(ot[:, ib, IS:1024], ops)
                    yield
            nc.sync.dma_start(out=out[:, it, :, :], in_=ot)

        ctxs_pair = None
        outgen = iter(())

        for it in range(NIS):
            njc = 4 * it + 4
            new_ctxs = [None, None]
            ctx_ps = {}
            pend = []

            def drain(it=it, njc=njc, new_ctxs=new_ctxs, ctx_ps=ctx_ps):
                h, q0, exps = pend.pop(0)
                for qi in range(2):
                    jc = q0 + qi
                    c0 = max(0, 128 * (jc - 4 * it))
                    nc.tensor.matmul(
                        ctx_ps[h][0:65, c0:IS],
                        v4[:, jc, h, :],
                        exps[:, qi, c0:IS],
                        start=(jc == 0), stop=(jc == njc - 1))
                if q0 + 2 == njc:
                    # head complete: 1/l, broadcast, normalize -> ctxs SBUF
                    pair, half = divmod(h, 2)
                    nc.vector.tensor_copy(lrow[0:1, h, :], ctx_ps[h][64:65, :])
                    nc.vector.reciprocal_approx_fast(rl[0:1, h, :],
                                                     lrow[0:1, h, :])
                    if half == 0:
                        new_ctxs[pair] = ctxp.tile([128, IS], BF16,
                                                   tag="c", name="ctxs")
                    bcs = bcp.tile([64, IS], F32, tag="bc", name="bcs")
                    nc.gpsimd.partition_broadcast(bcs, rl[0:1, h, :])
                    nc.vector.tensor_mul(
                        new_ctxs[pair][64 * half:64 * half + 64, :],
                        ctx_ps[h][0:64, :], bcs)

            for h in range(4):
                pair, half = divmod(h, 2)
                hb = 64 * half
                qs = qt[pair][hb:hb + 64, :]
                ks = kt[pair][hb:hb + 64, :]
                ctx_ps[h] = psum.tile([128, IS], F32, tag="cx", name=f"ctx{h}")
                for q0 in range(0, njc, 2):
                    quad = psum.tile([128, 2, IS], F32, tag="b", name="quad")
                    exps = expp.tile([128, 2, IS], BF16, tag="e", name="exps")
                    trim0 = max(0, 128 * (q0 - 4 * it))
                    for qi in range(2):
                        jc = q0 + qi
                        c0 = max(0, 128 * (jc - 4 * it))
                        nc.tensor.matmul(
                            quad[:, qi, c0:IS],
                            ks[:, 128 * jc:128 * (jc + 1)],
                            qs[:, it * IS + c0:(it + 1) * IS],
                            start=True, stop=True)
                    nc.scalar.activation(
                        exps[:, 0:2, trim0:IS], quad[:, 0:2, trim0:IS],
                        mybir.ActivationFunctionType.Exp, scale=0.125)
                    if q0 >= 4 * it:
                        # both chunks diagonal: one fused causal-mask multiply
                        # over [qi, trim+128qi : trim+128qi+128] (stride 640)
                        m = bass.AP(tensor=exps.tensor,
                                    offset=exps.offset + trim0,
                                    ap=[exps.ap[0], [IS + 128, 2], [1, 128]])
                        nc.vector.tensor_mul(m, m, tri2_sb)
                    next(outgen, None)
                    if pend:
                        drain()
                    pend.append((h, q0, exps))
            while pend:
                drain()
            for _ in outgen:
                pass
            ctxs_pair = new_ctxs
            outgen = outproj_gen(ctxs_pair, it)
        for _ in outgen:
            pass

    nc.compile()
    return nc


def _host_tables():
    inv_freq = 1.0 / (THETA ** (np.arange(0, HD, 2, dtype=np.float64) / HD))
    pos = np.arange(S, dtype=np.float64)
    ang = pos[None, :] * inv_freq[:, None]          # [32, S]
    cos32 = np.cos(ang).astype(np.float32)
    sin32 = np.sin(ang).astype(np.float32)
    cdup = np.concatenate([cos32, cos32, cos32, cos32], axis=0)  # [128, S]
    s_signed = np.concatenate([-sin32, sin32, -sin32, sin32], axis=0)
    p64 = np.zeros((128, 128), dtype=np.float32)
    for m in range(128):
        blk = m - (m % 64)
        d = m % 64
        p64[blk + ((d + 32) % 64), m] = 1.0
    tri = (np.arange(128)[:, None] <= np.arange(128)[None, :]).astype(np.float32)
    tri2 = np.stack([tri, tri], axis=1)  # [128, 2, 128]
    return cdup, s_signed, p64, tri2


_NC_CACHE = {}


def make_in_maps(x, W_q, W_k, W_v, W_o):
    cdup, sdup, p64, tri2 = _host_tables()
    ones = np.ones((128, 65), dtype=BF)
    def wshuf(w):  # [1024, 256] -> [128, 8, 256]
        return np.ascontiguousarray(
            w.reshape(8, 128, 256).transpose(1, 0, 2)).astype(BF)

    in_maps = []
    for c in range(NCORES):
        b, g = divmod(c, 4)
        cols = slice(256 * g, 256 * (g + 1))
        # xT[p, it, c, i] = x[b][512it+i, 128c+p]
        xtr = np.ascontiguousarray(
            x[b].reshape(NIS, IS, 8, 128).transpose(3, 0, 2, 1)).astype(BF)
        in_maps.append({
            "xT": xtr,
            "wq": wshuf(W_q[:, cols]),
            "wk": wshuf(W_k[:, cols]),
            "wv": wshuf(W_v[:, cols]),
            "wo": np.ascontiguousarray(
                W_o[cols, :].reshape(2, 128, 1024).transpose(1, 0, 2)).astype(BF),
            "cdup": cdup, "sdup": sdup,
            "p64": p64.astype(BF), "tri2": tri2.astype(BF),
            "onesc": ones,
        })
    return in_maps


def kernel(x, W_q, W_k, W_v, W_o):
    x = np.ascontiguousarray(x, dtype=np.float32)
    W_q = np.ascontiguousarray(W_q, dtype=np.float32)
    W_k = np.ascontiguousarray(W_k, dtype=np.float32)
    W_v = np.ascontiguousarray(W_v, dtype=np.float32)
    W_o = np.ascontiguousarray(W_o, dtype=np.float32)

    if "nc" not in _NC_CACHE:
        _NC_CACHE["nc"] = build_kernel()
    nc = _NC_CACHE["nc"]

    in_maps = make_in_maps(x, W_q, W_k, W_v, W_o)
    res = run_bass_kernel_spmd(nc, in_maps, list(range(NCORES)))
    full = np.zeros((B, S, D_OUT), dtype=np.float32)
    for b in range(B):
        for g in range(4):
            arr = np.asarray(res.results[4 * b + g]["out"], dtype=np.float32)
            # arr[p, it, ib, n] -> row 512it+128ib+p
            full[b] += arr.transpose(1, 2, 0, 3).reshape(S, D_OUT)
    return full


# revision 15
# speedup vs baseline: 1.3474x; 1.0131x over previous
"""Multi-head attention (RoPE, causal) Bass kernel for 8 TRN2 NeuronCores.

Sharding: 2-way batch x 4-way heads (4 heads per core); per-core partial
out[2048, 1024] summed on host (replaces the W_o-row-parallel AllReduce).

Fused schedule (per core): one software-pipelined stream per i-slice
"window".  Window `it` runs the attention of slice `it` — scores ->
exp (Act) -> causal mask (DVE) -> ctx matmul deferred one chunk-pair — and
pulls interleaved PE "bites" from a feed generator carrying the previous
slice's out-projection and the NEXT slice's QKV projections + RoPE, so the
PE never idles while Act chews the exps.  QT/KT stacked [128, S] bf16:
partitions 0-63 head even, 64-127 head odd.  Softmax denominator rides as
a 65th ones-column on V (PSUM row 64); 1/l via custom-DVE fast reciprocal
off a partition-0 staging row.  Everything bf16 except PSUM, RoPE tables
and the l/norm path (fp32).

PSUM (8 banks): tag "b" 2x[128,2,512] quads (4), tag "cx" 2x ctx
accumulators (2), tag "op" 2x shared proj/perm/vps/out-proj (2).
"""
import numpy as np
import ml_dtypes
from contextlib import ExitStack
from itertools import chain as ichain

import concourse.bass as bass
import concourse.tile as tile
from concourse import bacc, mybir
from concourse.bass_utils import run_bass_kernel_spmd

D_IN = 1024
D_OUT = 1024
HD = 64                   # head dim
S = 2048                  # sequence length
B = 2
THETA = 10000.0
NCORES = 8
IS = 512                  # i-slice width
NIS = S // IS             # 4 i-slices
NJC = S // 128            # 16 j-chunks

F32 = mybir.dt.float32
BF16 = mybir.dt.bfloat16
BF = ml_dtypes.bfloat16


def build_kernel():
    nc = bacc.Bacc("TRN2", target_bir_lowering=False, debug=False)

    # host pre-shuffled so every DMA is contiguous per partition:
    # xT[p, it, c, i] = x[b][512it+i, 128c+p]; w*[p, c, n] = W[128c+p, n]
    xT = nc.dram_tensor("xT", [128, NIS, 8, IS], BF16, kind="ExternalInput").ap()
    wq = nc.dram_tensor("wq", [128, 8, 256], BF16, kind="ExternalInput").ap()
    wk = nc.dram_tensor("wk", [128, 8, 256], BF16, kind="ExternalInput").ap()
    wv = nc.dram_tensor("wv", [128, 8, 256], BF16, kind="ExternalInput").ap()
    wo = nc.dram_tensor("wo", [128, 2, 1024], BF16, kind="ExternalInput").ap()
    cdup = nc.dram_tensor("cdup", [128, S], F32, kind="ExternalInput").ap()
    sdup = nc.dram_tensor("sdup", [128, S], F32, kind="ExternalInput").ap()
    p64 = nc.dram_tensor("p64", [128, 128], BF16, kind="ExternalInput").ap()
    tri2 = nc.dram_tensor("tri2", [128, 2, 128], BF16, kind="ExternalInput").ap()
    onesc = nc.dram_tensor("onesc", [128, 65], BF16, kind="ExternalInput").ap()
    # out[p, it, ib, n] = row 512it+128ib+p of the [2048, 1024] partial
    out = nc.dram_tensor("out", [128, NIS, 4, 1024], BF16,
                         kind="ExternalOutput").ap()

    with tile.TileContext(nc) as tc, ExitStack() as ctx:
        singles = ctx.enter_context(tc.tile_pool(name="singles", bufs=1))
        xpool = ctx.enter_context(tc.tile_pool(name="xpool", bufs=2))
        rope_tmp = ctx.enter_context(tc.tile_pool(name="rope_tmp", bufs=3))
        expp = ctx.enter_context(tc.tile_pool(name="expp", bufs=3))
        bcp = ctx.enter_context(tc.tile_pool(name="bcp", bufs=2))
        ctxp = ctx.enter_context(tc.tile_pool(name="ctxp", bufs=3))
        outp = ctx.enter_context(tc.tile_pool(name="outp", bufs=2))
        psum = ctx.enter_context(tc.tile_pool(name="psum", bufs=2, space="PSUM"))

        # ---- DMAs, ordered by first PE use ----
        wq_t = singles.tile([128, 8, 256], BF16, tag="wq", name="wq")
        xt0_t = xpool.tile([128, 8, IS], BF16, tag="xt", name="xt0")
        for c in range(2):
            nc.sync.dma_start(out=wq_t[:, c, :], in_=wq[:, c, :])
            nc.sync.dma_start(out=xt0_t[:, c, :], in_=xT[:, 0, c, :])
        xts = {0: xt0_t}

        def xt_dma(it):
            t = xpool.tile([128, 8, IS], BF16, tag="xt", name=f"xt{it}")
            nc.sync.dma_start(out=t, in_=xT[:, it, :, :])
            xts[it] = t

        xt_dma(1)
        for c in range(2, 8):
            nc.sync.dma_start(out=wq_t[:, c, :], in_=wq[:, c, :])
            nc.sync.dma_start(out=xt0_t[:, c, :], in_=xT[:, 0, c, :])
        c_sb = singles.tile([128, S], F32, tag="cdup")
        nc.sync.dma_start(out=c_sb[:, 0:1024], in_=cdup[:, 0:1024])
        wk_t = singles.tile([128, 8, 256], BF16, tag="wk", name="wk")
        nc.sync.dma_start(out=wk_t[:, 0:4, :], in_=wk[:, 0:4, :])
        nc.sync.dma_start(out=wk_t[:, 4:8, :], in_=wk[:, 4:8, :])
        s_sb = singles.tile([128, S], F32, tag="sdup")
        nc.sync.dma_start(out=s_sb[:, 0:1024], in_=sdup[:, 0:1024])
        p64_sb = singles.tile([128, 128], BF16, tag="p64")
        nc.sync.dma_start(out=p64_sb, in_=p64)
        wv_t = singles.tile([128, 8, 256], BF16, tag="wv", name="wv")
        nc.sync.dma_start(out=wv_t, in_=wv)
        onesc_sb = singles.tile([128, 65], BF16, tag="ones")
        nc.sync.dma_start(out=onesc_sb, in_=onesc)
        tri2_sb = singles.tile([128, 2, 128], BF16, tag="tri2")
        nc.sync.dma_start(out=tri2_sb, in_=tri2)
        nc.sync.dma_start(out=c_sb[:, 1024:2048], in_=cdup[:, 1024:2048])
        nc.sync.dma_start(out=s_sb[:, 1024:2048], in_=sdup[:, 1024:2048])
        # deferred: wo is not needed until the first out-projection
        wo_sb = singles.tile([128, 2, 1024], BF16, tag="wo")

        # persistent SBUF state
        qt = [singles.tile([128, S], BF16, tag=f"qt{p}", name=f"qt{p}")
              for p in range(2)]
        kt = [singles.tile([128, S], BF16, tag=f"kt{p}", name=f"kt{p}")
              for p in range(2)]
        v4 = singles.tile([128, NJC, 4, 65], BF16, tag="v4")
        v4_ones = bass.AP(tensor=v4.tensor, offset=64,
                          ap=[[NJC * 4 * 65, 128], [65, NJC * 4]])
        # 1/l rows, all on partition 0: [h, i] along the free dim.
        # lrow stages the PSUM row-64 l to partition 0 first: the custom-DVE
        # reciprocal mishandles nonzero input partition offsets.
        rl = singles.tile([128, 4, IS], F32, tag="rl")
        lrow = singles.tile([128, 4, IS], F32, tag="lrow")

        # ---------------- QKV projection + RoPE bites -----------------
        def rope_stage1(proj, key, it):
            raw = rope_tmp.tile([128, IS], BF16, tag="raw", name=f"raw{key}")
            nc.scalar.copy(raw, proj)
            t1 = rope_tmp.tile([128, IS], F32, tag="t1", name=f"t1{key}")
            nc.vector.tensor_mul(t1, proj, c_sb[:, it * IS:(it + 1) * IS])
            return raw, t1

        def rope_stage2(raw, t1, dest, it):
            perm = psum.tile([128, IS], F32, tag="op", name="perm")
            nc.tensor.matmul(perm, p64_sb, raw, start=True, stop=True)
            t2 = rope_tmp.tile([128, IS], F32, tag="t2")
            nc.vector.tensor_mul(t2, perm, s_sb[:, it * IS:(it + 1) * IS])
            nc.vector.tensor_add(dest, t1, t2)

        def qkv_bites(it):
            """QKV projections + RoPE for slice `it`, yielded in ~2-matmul
            bites so they interleave into the attention pipeline."""
            if it >= NIS:
                return
            if 2 <= it + 1 < NIS:
                xt_dma(it + 1)
            xt_t = xts[it]
            pending_rope = None
            for tname, wt, dests in (("q", wq_t, qt), ("k", wk_t, kt)):
                for p in range(2):
                    proj = psum.tile([128, IS], F32, tag="op", name="proj")
                    for c in range(0, 8, 2):
                        nc.tensor.matmul(proj, wt[:, c, 128 * p:128 * (p + 1)],
                                         xt_t[:, c, :],
                                         start=(c == 0), stop=False)
                        nc.tensor.matmul(proj, wt[:, c + 1, 128 * p:128 * (p + 1)],
                                         xt_t[:, c + 1, :],
                                         start=False, stop=(c + 1 == 7))
                        yield
                    if pending_rope is not None:
                        rope_stage2(*pending_rope)
                    raw, t1 = rope_stage1(proj, f"{tname}{p}", it)
                    pending_rope = (raw, t1,
                                    dests[p][:, it * IS:(it + 1) * IS], it)
                    yield
            for half in range(2):
                vps = psum.tile([128, 2, 256], F32, tag="op", name="vps")
                for js in range(2):
                    for c in range(0, 8, 2):
                        for cc in (c, c + 1):
                            nc.tensor.matmul(
                                vps[:, js, :],
                                xt_t[:, cc, 128 * (half * 2 + js):128 * (half * 2 + js + 1)],
                                wv_t[:, cc, :],
                                start=(cc == 0), stop=(cc == 7))
                        yield
                if half == 0 and pending_rope is not None:
                    rope_stage2(*pending_rope)
                    pending_rope = None
                    yield
                for js in range(2):
                    jt = it * 4 + half * 2 + js
                    nc.vector.tensor_copy(v4[:, jt, :, 0:64],
                                          vps[:, js, :].rearrange("p (h d) -> p h d", h=4))

        def outproj_bites(ctxs_pair, it):
            """Out-projection of slice `it`: 8 bites of 2 matmuls each."""
            ot = outp.tile([128, 4, 1024], BF16, tag="o", name="ot")
            for ib in range(4):
                for nt in range(2):
                    ops = psum.tile([128, IS], F32, tag="op", name="ops")
                    for pair in range(2):
                        nc.tensor.matmul(
                            ops,
                            ctxs_pair[pair][:, 128 * ib:128 * (ib + 1)],
                            wo_sb[:, pair, nt * IS:(nt + 1) * IS],
                            start=(pair == 0), stop=(pair == 1))
                    # split the PSUM->SBUF copies across Act and DVE
                    if nt == 0:
                        nc.scalar.copy(ot[:, ib, 0:IS], ops)
                    else:
                        nc.vector.tensor_copy(ot[:, ib, IS:1024], ops)
                    yield
            nc.sync.dma_start(out=out[:, it, :, :], in_=ot)

        # ---------------- prologue: slice-0 projections --------------
        for _ in qkv_bites(0):
            pass
        nc.sync.dma_start(out=wo_sb, in_=wo)
        # ones column for the softmax denominator
        nc.vector.tensor_copy(v4_ones, onesc_sb[:, 0:64])

        # ------------- fused attention windows ------------------------
        ctxs_pair = None
        feed = qkv_bites(1)

        for it in range(NIS):
            njc = 4 * it + 4
            new_ctxs = [None, None]
            ctx_ps = {}
            pend = []

            def drain(it=it, njc=njc, new_ctxs=new_ctxs, ctx_ps=ctx_ps):
                h, q0, exps = pend.pop(0)
                for qi in range(2):
                    jc = q0 + qi
                    c0 = max(0, 128 * (jc - 4 * it))
                    nc.tensor.matmul(
                        ctx_ps[h][0:65, c0:IS],
                        v4[:, jc, h, :],
                        exps[:, qi, c0:IS],
                        start=(jc == 0), stop=(jc == njc - 1))
                if q0 + 2 == njc:
                    # head complete: 1/l, broadcast, normalize -> ctxs SBUF
                    pair, half = divmod(h, 2)
                    nc.vector.tensor_copy(lrow[0:1, h, :], ctx_ps[h][64:65, :])
                    nc.vector.reciprocal_approx_fast(rl[0:1, h, :],
                                                     lrow[0:1, h, :])
                    if half == 0:
                        new_ctxs[pair] = ctxp.tile([128, IS], BF16,
                                                   tag="c", name="ctxs")
                    bcs = bcp.tile([64, IS], F32, tag="bc", name="bcs")
                    nc.gpsimd.partition_broadcast(bcs, rl[0:1, h, :])
                    nc.vector.tensor_mul(
                        new_ctxs[pair][64 * half:64 * half + 64, :],
                        ctx_ps[h][0:64, :], bcs)

            for h in range(4):
                pair, half = divmod(h, 2)
                hb = 64 * half
                qs = qt[pair][hb:hb + 64, :]
                ks = kt[pair][hb:hb + 64, :]
                ctx_ps[h] = psum.tile([128, IS], F32, tag="cx", name=f"ctx{h}")
                for q0 in range(0, njc, 2):
                    quad = psum.tile([128, 2, IS], F32, tag="b", name="quad")
                    exps = expp.tile([128, 2, IS], BF16, tag="e", name="exps")
                    trim0 = max(0, 128 * (q0 - 4 * it))
                    for qi in range(2):
                        jc = q0 + qi
                        c0 = max(0, 128 * (jc - 4 * it))
                        nc.tensor.matmul(
                            quad[:, qi, c0:IS],
                            ks[:, 128 * jc:128 * (jc + 1)],
                            qs[:, it * IS + c0:(it + 1) * IS],
                            start=True, stop=True)
                    nc.scalar.activation(
                        exps[:, 0:2, trim0:IS], quad[:, 0:2, trim0:IS],
                        mybir.ActivationFunctionType.Exp, scale=0.125)
                    if q0 >= 4 * it:
                        # both chunks diagonal: one fused causal-mask multiply
                        m = bass.AP(tensor=exps.tensor,
                                    offset=exps.offset + trim0,
                                    ap=[exps.ap[0], [IS + 128, 2], [1, 128]])
                        nc.vector.tensor_mul(m, m, tri2_sb)
                    next(feed, None)
                    if pend:
                        drain()
                    pend.append((h, q0, exps))
            while pend:
                drain()
            for _ in feed:
                pass
            ctxs_pair = new_ctxs
            feed = ichain(outproj_bites(ctxs_pair, it), qkv_bites(it + 2))
        for _ in feed:
            pass

    nc.compile()
    return nc


def _host_tables():
    inv_freq = 1.0 / (THETA ** (np.arange(0, HD, 2, dtype=np.float64) / HD))
    pos = np.arange(S, dtype=np.float64)
    ang = pos[None, :] * inv_freq[:, None]          # [32, S]
    cos32 = np.cos(ang).astype(np.float32)
    sin32 = np.sin(ang).astype(np.float32)
    cdup = np.concatenate([cos32, cos32, cos32, cos32], axis=0)  # [128, S]
    s_signed = np.concatenate([-sin32, sin32, -sin32, sin32], axis=0)
    p64 = np.zeros((128, 128), dtype=np.float32)
    for m in range(128):
        blk = m - (m % 64)
        d = m % 64
        p64[blk + ((d + 32) % 64), m] = 1.0
    tri = (np.arange(128)[:, None] <= np.arange(128)[None, :]).astype(np.float32)
    tri2 = np.stack([tri, tri], axis=1)  # [128, 2, 128]
    return cdup, s_signed, p64, tri2


_NC_CACHE = {}


def make_in_maps(x, W_q, W_k, W_v, W_o):
    cdup, sdup, p64, tri2 = _host_tables()
    ones = np.ones((128, 65), dtype=BF)
    def wshuf(w):  # [1024, 256] -> [128, 8, 256]
        return np.ascontiguousarray(
            w.reshape(8, 128, 256).transpose(1, 0, 2)).astype(BF)

    in_maps = []
    for c in range(NCORES):
        b, g = divmod(c, 4)
        cols = slice(256 * g, 256 * (g + 1))
        # xT[p, it, c, i] = x[b][512it+i, 128c+p]
        xtr = np.ascontiguousarray(
            x[b].reshape(NIS, IS, 8, 128).transpose(3, 0, 2, 1)).astype(BF)
        in_maps.append({
            "xT": xtr,
            "wq": wshuf(W_q[:, cols]),
            "wk": wshuf(W_k[:, cols]),
            "wv": wshuf(W_v[:, cols]),
            "wo": np.ascontiguousarray(
                W_o[cols, :].reshape(2, 128, 1024).transpose(1, 0, 2)).astype(BF),
            "cdup": cdup, "sdup": sdup,
            "p64": p64.astype(BF), "tri2": tri2.astype(BF),
            "onesc": ones,
        })
    return in_maps


def kernel(x, W_q, W_k, W_v, W_o):
    x = np.ascontiguousarray(x, dtype=np.float32)
    W_q = np.ascontiguousarray(W_q, dtype=np.float32)
    W_k = np.ascontiguousarray(W_k, dtype=np.float32)
    W_v = np.ascontiguousarray(W_v, dtype=np.float32)
    W_o = np.ascontiguousarray(W_o, dtype=np.float32)

    if "nc" not in _NC_CACHE:
        _NC_CACHE["nc"] = build_kernel()
    nc = _NC_CACHE["nc"]

    in_maps = make_in_maps(x, W_q, W_k, W_v, W_o)
    res = run_bass_kernel_spmd(nc, in_maps, list(range(NCORES)))
    full = np.zeros((B, S, D_OUT), dtype=np.float32)
    for b in range(B):
        for g in range(4):
            arr = np.asarray(res.results[4 * b + g]["out"], dtype=np.float32)
            # arr[p, it, ib, n] -> row 512it+128ib+p
            full[b] += arr.transpose(1, 2, 0, 3).reshape(S, D_OUT)
    return full


# revision 27
# speedup vs baseline: 1.4321x; 1.0629x over previous
"""Multi-head attention (RoPE, causal) Bass kernel for 8 TRN2 NeuronCores.

Sharding: 2-way batch x 4-way heads (4 heads per core); per-core partial
out[2048, 1024] summed on host (replaces the W_o-row-parallel AllReduce).

Fused schedule (per core): one software-pipelined stream per i-slice
"window".  Window `it` runs the attention of slice `it` — scores ->
exp (Act) -> causal mask (DVE) -> ctx matmul deferred one chunk-pair — and
pulls interleaved PE "bites" from a feed generator carrying the previous
slice's out-projection and the NEXT slice's QKV projections + RoPE, so the
PE never idles while Act chews the exps.  QT/KT stacked [128, S] bf16:
partitions 0-63 head even, 64-127 head odd.  Softmax denominator rides as
a 65th ones-column on V (PSUM row 64); 1/l via custom-DVE fast reciprocal
off a partition-0 staging row.  Everything bf16 except PSUM, RoPE tables
and the l/norm path (fp32).

PSUM (8 banks): tag "b" 2x[128,2,512] quads (4), tag "cx" 2x ctx
accumulators (2), tag "op" 2x shared proj/perm/vps/out-proj (2).
"""
import numpy as np
import ml_dtypes
from contextlib import ExitStack
from itertools import chain as ichain

import concourse.bass as bass
import concourse.tile as tile
from concourse import bacc, mybir
from concourse.bass_utils import run_bass_kernel_spmd

D_IN = 1024
D_OUT = 1024
HD = 64                   # head dim
S = 2048                  # sequence length
B = 2
THETA = 10000.0
NCORES = 8
IS = 512                  # i-slice width
NIS = S // IS             # 4 i-slices
NJC = S // 128            # 16 j-chunks

F32 = mybir.dt.float32
BF16 = mybir.dt.bfloat16
BF = ml_dtypes.bfloat16


def build_kernel():
    nc = bacc.Bacc("TRN2", target_bir_lowering=False, debug=False)

    # host pre-shuffled so every DMA is contiguous per partition:
    # xT[p, it, c, i] = x[b][512it+i, 128c+p]; w*[p, c, n] = W[128c+p, n]
    xT = nc.dram_tensor("xT", [128, NIS, 8, IS], BF16, kind="ExternalInput").ap()
    wq = nc.dram_tensor("wq", [128, 8, 256], BF16, kind="ExternalInput").ap()
    wk = nc.dram_tensor("wk", [128, 8, 256], BF16, kind="ExternalInput").ap()
    wv = nc.dram_tensor("wv", [128, 8, 256], BF16, kind="ExternalInput").ap()
    wo = nc.dram_tensor("wo", [128, 2, 1024], BF16, kind="ExternalInput").ap()
    cdup = nc.dram_tensor("cdup", [128, S], BF16, kind="ExternalInput").ap()
    sdup = nc.dram_tensor("sdup", [128, S], BF16, kind="ExternalInput").ap()
    tri2 = nc.dram_tensor("tri2", [128, 2, 128], BF16, kind="ExternalInput").ap()
    onesc = nc.dram_tensor("onesc", [128, 65], BF16, kind="ExternalInput").ap()
    # out[p, it, ib, n] = row 512it+128ib+p of the [2048, 1024] partial
    out = nc.dram_tensor("out", [128, NIS, 4, 1024], BF16,
                         kind="ExternalOutput").ap()

    with tile.TileContext(nc) as tc, ExitStack() as ctx:
        singles = ctx.enter_context(tc.tile_pool(name="singles", bufs=1))
        xpool = ctx.enter_context(tc.tile_pool(name="xpool", bufs=2))
        rope_tmp = ctx.enter_context(tc.tile_pool(name="rope_tmp", bufs=3))
        expp = ctx.enter_context(tc.tile_pool(name="expp", bufs=4))
        bcp = ctx.enter_context(tc.tile_pool(name="bcp", bufs=2))
        ctxp = ctx.enter_context(tc.tile_pool(name="ctxp", bufs=3))
        outp = ctx.enter_context(tc.tile_pool(name="outp", bufs=2))
        psum = ctx.enter_context(tc.tile_pool(name="psum", bufs=2, space="PSUM"))

        # ---- DMAs, ordered by first PE use ----
        wq_t = singles.tile([128, 8, 256], BF16, tag="wq", name="wq")
        xt0_t = xpool.tile([128, 8, IS], BF16, tag="xt", name="xt0")
        for c in range(2):
            nc.sync.dma_start(out=wq_t[:, c, :], in_=wq[:, c, :])
            nc.sync.dma_start(out=xt0_t[:, c, :], in_=xT[:, 0, c, :])
        xts = {0: xt0_t}

        def xt_dma(it):
            t = xpool.tile([128, 8, IS], BF16, tag="xt", name=f"xt{it}")
            nc.sync.dma_start(out=t, in_=xT[:, it, :, :])
            xts[it] = t

        for c in range(2, 8):
            nc.sync.dma_start(out=wq_t[:, c, :], in_=wq[:, c, :])
            nc.sync.dma_start(out=xt0_t[:, c, :], in_=xT[:, 0, c, :])
        wk_t = singles.tile([128, 8, 256], BF16, tag="wk", name="wk")
        nc.sync.dma_start(out=wk_t[:, 0:4, :], in_=wk[:, 0:4, :])
        nc.sync.dma_start(out=wk_t[:, 4:8, :], in_=wk[:, 4:8, :])
        c_sb = singles.tile([128, S], BF16, tag="cdup")
        nc.sync.dma_start(out=c_sb, in_=cdup)
        xt_dma(1)
        s_sb = singles.tile([128, S], BF16, tag="sdup")
        nc.sync.dma_start(out=s_sb, in_=sdup)
        wv_t = singles.tile([128, 8, 256], BF16, tag="wv", name="wv")
        nc.sync.dma_start(out=wv_t, in_=wv)
        onesc_sb = singles.tile([128, 65], BF16, tag="ones")
        nc.sync.dma_start(out=onesc_sb, in_=onesc)
        tri2_sb = singles.tile([128, 2, 128], BF16, tag="tri2")
        nc.sync.dma_start(out=tri2_sb, in_=tri2)
        # deferred: wo is not needed until the first out-projection
        wo_sb = singles.tile([128, 2, 1024], BF16, tag="wo")

        # persistent SBUF state
        qt = [singles.tile([128, S], BF16, tag=f"qt{p}", name=f"qt{p}")
              for p in range(2)]
        kt = [singles.tile([128, S], BF16, tag=f"kt{p}", name=f"kt{p}")
              for p in range(2)]
        v4 = singles.tile([128, NJC, 4, 65], BF16, tag="v4")
        v4_ones = bass.AP(tensor=v4.tensor, offset=64,
                          ap=[[NJC * 4 * 65, 128], [65, NJC * 4]])
        # 1/l rows, all on partition 0: [h, i] along the free dim.
        # lrow stages the PSUM row-64 l to partition 0 first: the custom-DVE
        # reciprocal mishandles nonzero input partition offsets.
        rl = singles.tile([128, 4, IS], F32, tag="rl")
        lrow = singles.tile([128, 4, IS], F32, tag="lrow")

        # ---------------- QKV projection + RoPE bites -----------------
        # rotate-half is a blockwise +-32 partition rotation: done with 4
        # SBUF->SBUF partition-range DMA copies instead of a PE perm matmul.
        def rope_stage1(proj, key, it):
            raw = rope_tmp.tile([128, IS], BF16, tag="raw", name=f"raw{key}")
            nc.scalar.copy(raw, proj)
            t1 = rope_tmp.tile([128, IS], BF16, tag="t1", name=f"t1{key}")
            nc.vector.tensor_mul(t1, raw, c_sb[:, it * IS:(it + 1) * IS])
            return raw, t1

        def rope_stage2(raw, t1, dest, it):
            permr = rope_tmp.tile([128, IS], BF16, tag="pr", name="permr")
            for a, b in ((0, 32), (32, 0), (64, 96), (96, 64)):
                nc.sync.dma_start(out=permr[a:a + 32, :], in_=raw[b:b + 32, :])
            t2 = rope_tmp.tile([128, IS], BF16, tag="t2")
            nc.vector.tensor_mul(t2, permr, s_sb[:, it * IS:(it + 1) * IS])
            nc.vector.tensor_add(dest, t1, t2)

        def qkv_bites(it):
            """QKV projections + RoPE for slice `it`, yielded in ~2-matmul
            bites so they interleave into the attention pipeline."""
            if it >= NIS:
                return
            if 2 <= it + 1 < NIS:
                xt_dma(it + 1)
            xt_t = xts[it]
            pending_rope = None
            for tname, wt, dests in (("q", wq_t, qt), ("k", wk_t, kt)):
                for p in range(2):
                    proj = psum.tile([128, IS], F32, tag="op", name="proj")
                    for c in range(0, 8, 2):
                        nc.tensor.matmul(proj, wt[:, c, 128 * p:128 * (p + 1)],
                                         xt_t[:, c, :],
                                         start=(c == 0), stop=False)
                        nc.tensor.matmul(proj, wt[:, c + 1, 128 * p:128 * (p + 1)],
                                         xt_t[:, c + 1, :],
                                         start=False, stop=(c + 1 == 7))
                        yield
                    if pending_rope is not None:
                        rope_stage2(*pending_rope)
                    raw, t1 = rope_stage1(proj, f"{tname}{p}", it)
                    pending_rope = (raw, t1,
                                    dests[p][:, it * IS:(it + 1) * IS], it)
            for half in range(2):
                vps = psum.tile([128, 2, 256], F32, tag="op", name="vps")
                for js in range(2):
                    for c in range(0, 8, 2):
                        for cc in (c, c + 1):
                            nc.tensor.matmul(
                                vps[:, js, :],
                                xt_t[:, cc, 128 * (half * 2 + js):128 * (half * 2 + js + 1)],
                                wv_t[:, cc, :],
                                start=(cc == 0), stop=(cc == 7))
                        yield
                if half == 0 and pending_rope is not None:
                    rope_stage2(*pending_rope)
                    pending_rope = None
                for js in range(2):
                    jt = it * 4 + half * 2 + js
                    nc.vector.tensor_copy(v4[:, jt, :, 0:64],
                                          vps[:, js, :].rearrange("p (h d) -> p h d", h=4))

        def outproj_bites(ctxs_pair, it):
            """Out-projection of slice `it`: 8 bites of 2 matmuls each."""
            ot = outp.tile([128, 4, 1024], BF16, tag="o", name="ot")
            for ib in range(4):
                for nt in range(2):
                    ops = psum.tile([128, IS], F32, tag="op", name="ops")
                    for pair in range(2):
                        nc.tensor.matmul(
                            ops,
                            ctxs_pair[pair][:, 128 * ib:128 * (ib + 1)],
                            wo_sb[:, pair, nt * IS:(nt + 1) * IS],
                            start=(pair == 0), stop=(pair == 1))
                    # PSUM->SBUF copies on DVE: Act keeps only exp + raw
                    nc.vector.tensor_copy(ot[:, ib, nt * IS:(nt + 1) * IS], ops)
                    yield
            nc.sync.dma_start(out=out[:, it, :, :], in_=ot)

        # ---------------- prologue: slice-0 projections --------------
        for _ in qkv_bites(0):
            pass
        nc.sync.dma_start(out=wo_sb, in_=wo)
        # ones column for the softmax denominator
        nc.vector.tensor_copy(v4_ones, onesc_sb[:, 0:64])

        # ------------- fused attention windows ------------------------
        ctxs_pair = None
        feed = qkv_bites(1)

        for it in range(NIS):
            njc = 4 * it + 4
            new_ctxs = [None, None]
            ctx_ps = {}
            pend = []

            def drain(it=it, njc=njc, new_ctxs=new_ctxs, ctx_ps=ctx_ps):
                h, q0, exps = pend.pop(0)
                for qi in range(2):
                    jc = q0 + qi
                    c0 = max(0, 128 * (jc - 4 * it))
                    nc.tensor.matmul(
                        ctx_ps[h][0:65, c0:IS],
                        v4[:, jc, h, :],
                        exps[:, qi, c0:IS],
                        start=(jc == 0), stop=(jc == njc - 1))
                if q0 + 2 == njc:
                    # head complete: 1/l, broadcast, normalize -> ctxs SBUF
                    pair, half = divmod(h, 2)
                    nc.vector.tensor_copy(lrow[0:1, h, :], ctx_ps[h][64:65, :])
                    nc.vector.reciprocal_approx_fast(rl[0:1, h, :],
                                                     lrow[0:1, h, :])
                    if half == 0:
                        new_ctxs[pair] = ctxp.tile([128, IS], BF16,
                                                   tag="c", name="ctxs")
                    bcs = bcp.tile([64, IS], F32, tag="bc", name="bcs")
                    nc.gpsimd.partition_broadcast(bcs, rl[0:1, h, :])
                    nc.vector.tensor_mul(
                        new_ctxs[pair][64 * half:64 * half + 64, :],
                        ctx_ps[h][0:64, :], bcs)

            for h in range(4):
                pair, half = divmod(h, 2)
                hb = 64 * half
                qs = qt[pair][hb:hb + 64, :]
                ks = kt[pair][hb:hb + 64, :]
                ctx_ps[h] = psum.tile([128, IS], F32, tag="cx", name=f"ctx{h}")
                for q0 in range(0, njc, 2):
                    quad = psum.tile([128, 2, IS], F32, tag="b", name="quad")
                    exps = expp.tile([128, 2, IS], BF16, tag="e", name="exps")
                    trim0 = max(0, 128 * (q0 - 4 * it))
                    for qi in range(2):
                        jc = q0 + qi
                        c0 = max(0, 128 * (jc - 4 * it))
                        nc.tensor.matmul(
                            quad[:, qi, c0:IS],
                            ks[:, 128 * jc:128 * (jc + 1)],
                            qs[:, it * IS + c0:(it + 1) * IS],
                            start=True, stop=True)
                    nc.scalar.activation(
                        exps[:, 0:2, trim0:IS], quad[:, 0:2, trim0:IS],
                        mybir.ActivationFunctionType.Exp, scale=0.125)
                    if q0 >= 4 * it:
                        # both chunks diagonal: one fused causal-mask multiply
                        m = bass.AP(tensor=exps.tensor,
                                    offset=exps.offset + trim0,
                                    ap=[exps.ap[0], [IS + 128, 2], [1, 128]])
                        nc.vector.tensor_mul(m, m, tri2_sb)
                    next(feed, None)
                    if len(pend) >= 2:
                        drain()
                    pend.append((h, q0, exps))
            while pend:
                drain()
            for _ in feed:
                pass
            ctxs_pair = new_ctxs
            feed = ichain(outproj_bites(ctxs_pair, it), qkv_bites(it + 2))
        for _ in feed:
            pass

    nc.compile()
    return nc


def _host_tables():
    inv_freq = 1.0 / (THETA ** (np.arange(0, HD, 2, dtype=np.float64) / HD))
    pos = np.arange(S, dtype=np.float64)
    ang = pos[None, :] * inv_freq[:, None]          # [32, S]
    cos32 = np.cos(ang).astype(np.float32)
    sin32 = np.sin(ang).astype(np.float32)
    cdup = np.concatenate([cos32, cos32, cos32, cos32], axis=0)  # [128, S]
    s_signed = np.concatenate([-sin32, sin32, -sin32, sin32], axis=0)
    tri = (np.arange(128)[:, None] <= np.arange(128)[None, :]).astype(np.float32)
    tri2 = np.stack([tri, tri], axis=1)  # [128, 2, 128]
    return cdup, s_signed, tri2


_NC_CACHE = {}


def make_in_maps(x, W_q, W_k, W_v, W_o):
    cdup, sdup, tri2 = _host_tables()
    ones = np.ones((128, 65), dtype=BF)
    def wshuf(w):  # [1024, 256] -> [128, 8, 256]
        return np.ascontiguousarray(
            w.reshape(8, 128, 256).transpose(1, 0, 2)).astype(BF)

    in_maps = []
    for c in range(NCORES):
        b, g = divmod(c, 4)
        cols = slice(256 * g, 256 * (g + 1))
        # xT[p, it, c, i] = x[b][512it+i, 128c+p]
        xtr = np.ascontiguousarray(
            x[b].reshape(NIS, IS, 8, 128).transpose(3, 0, 2, 1)).astype(BF)
        in_maps.append({
            "xT": xtr,
            "wq": wshuf(W_q[:, cols]),
            "wk": wshuf(W_k[:, cols]),
            "wv": wshuf(W_v[:, cols]),
            "wo": np.ascontiguousarray(
                W_o[cols, :].reshape(2, 128, 1024).transpose(1, 0, 2)).astype(BF),
            "cdup": cdup.astype(BF), "sdup": sdup.astype(BF),
            "tri2": tri2.astype(BF),
            "onesc": ones,
        })
    return in_maps


def kernel(x, W_q, W_k, W_v, W_o):
    x = np.ascontiguousarray(x, dtype=np.float32)
    W_q = np.ascontiguousarray(W_q, dtype=np.float32)
    W_k = np.ascontiguousarray(W_k, dtype=np.float32)
    W_v = np.ascontiguousarray(W_v, dtype=np.float32)
    W_o = np.ascontiguousarray(W_o, dtype=np.float32)

    if "nc" not in _NC_CACHE:
        _NC_CACHE["nc"] = build_kernel()
    nc = _NC_CACHE["nc"]

    in_maps = make_in_maps(x, W_q, W_k, W_v, W_o)
    res = run_bass_kernel_spmd(nc, in_maps, list(range(NCORES)))
    full = np.zeros((B, S, D_OUT), dtype=np.float32)
    for b in range(B):
        for g in range(4):
            arr = np.asarray(res.results[4 * b + g]["out"], dtype=np.float32)
            # arr[p, it, ib, n] -> row 512it+128ib+p
            full[b] += arr.transpose(1, 2, 0, 3).reshape(S, D_OUT)
    return full
